# revision 11
# baseline (speedup 1.0000x reference)
"""Swin-style windowed attention TRN2 kernel (v2).

Math per window (n=49 tokens, d=128, 4 heads x 32):
  qkv = x @ W_qkv ; q *= dh**-0.5
  sim[h] = q_h @ k_h^T + bias[h] ; attn = softmax_j(sim)
  out = (attn @ v) @ W_out

v2 layout (per QUAD = 4 windows = 2 pairs):
  - x pair [98,128] --PE transpose--> xT cols of [128, 196] (bf16)
  - qT/kT head-split [32, 4, 196] (sim operands need base partition 0)
  - sim psum [128, 392]: two pair-stacks side by side; within a stack,
    window A rows 0:49, window B rows 64:113 (no cross-window garbage)
  - per (stack, wblock): bias-init matmul at matching tile_position, then
    per-head sim matmuls accumulate (consistent position per group)
  - ONE exp per quad [128, 392] -> U bf16 ; DVE reduce -> s [128, 8]
  - reciprocal ; ONE normalize op with step-0 broadcast of r
  - 16 PE transposes -> UT [49, 16, 49] ; AV -> av [64, 2, 4, 49]
  - proj per pair: 4 accumulating K=32 matmuls -> [98, 128] -> DMA out
"""

import os
import numpy as np
import ml_dtypes

import concourse.bass as bass
import concourse.mybir as mybir
import concourse.tile as tile
from concourse import bacc
from concourse.bass_utils import run_bass_kernel_spmd

DIM = 128
DH = 32
HEADS = 4
WS = 7
N = 49
SCALE = DH ** -0.5
P = 98            # tokens per window pair
QW = 4            # windows per quad
QT = 196          # tokens per quad
NCORES = 8
GROUP = 8         # pairs per DMA group (= 4 quads)

F32 = mybir.dt.float32
BF16 = mybir.dt.bfloat16
BF = ml_dtypes.bfloat16


def _rel_pos_bias(bias_table):
    pos = np.arange(WS)
    gi, gj = np.meshgrid(pos, pos, indexing="ij")
    grid = np.stack([gi, gj], -1).reshape(N, 2)
    rel = grid[:, None, :] - grid[None, :, :] + (WS - 1)
    idx = rel[..., 0] * (2 * WS - 1) + rel[..., 1]          # [N, N] int
    b = np.asarray(bias_table, np.float32)[idx]             # [N, N, H]
    return np.transpose(b, (2, 0, 1))                       # [H, N, N]


def _build_bias_block(bias_table):
    """[64, 4*49]: rows i (49 real + 15 pad), cols (h, j)."""
    bh = _rel_pos_bias(bias_table)
    out = np.zeros((64, HEADS * N), np.float32)
    for h in range(HEADS):
        out[:N, N * h:N * h + N] = bh[h]
    return out


def build_program(n_pairs, group=GROUP, repeats=1):
    nc = bacc.Bacc("TRN2", target_bir_lowering=False)
    TOK = n_pairs * P
    n_groups = n_pairs // group
    assert n_pairs % group == 0 and group % 2 == 0
    quads_per_group = group // 2
    SKIP = os.environ.get("KSKIP", "").split(",")

    x_d = nc.declare_dram_parameter("x", [TOK, DIM], F32, isOutput=False)
    wq_d = nc.declare_dram_parameter("wq", [DIM, DIM], BF16, isOutput=False)
    wk_d = nc.declare_dram_parameter("wk", [DIM, DIM], BF16, isOutput=False)
    wv_d = nc.declare_dram_parameter("wv", [DIM, DIM], BF16, isOutput=False)
    wo_d = nc.declare_dram_parameter("wo", [64, 2, DIM], BF16, isOutput=False)
    bias_d = nc.declare_dram_parameter("biasb", [64, HEADS * N], BF16, isOutput=False)
    i98_d = nc.declare_dram_parameter("i98", [DIM, DIM], BF16, isOutput=False)
    out_d = nc.declare_dram_parameter("out", [TOK, DIM], BF16, isOutput=True)

    with tile.TileContext(nc) as tc:
        with (
            tc.tile_pool(name="const", bufs=1) as constp,
            tc.tile_pool(name="stage", bufs=4) as stagep,
            tc.tile_pool(name="xt", bufs=4) as xtp,
            tc.tile_pool(name="qk", bufs=4) as qkp,
            tc.tile_pool(name="vn", bufs=4) as vnp,
            tc.tile_pool(name="u", bufs=5) as up,
            tc.tile_pool(name="sr", bufs=6) as srp,
            tc.tile_pool(name="ut", bufs=4) as utp_pool,
            tc.tile_pool(name="ot", bufs=4) as otp,
            tc.tile_pool(name="fin", bufs=4) as finp,
            tc.tile_pool(name="psCD", bufs=int(os.environ.get("BCD", "3")),
                         space="PSUM") as psCD,
            tc.tile_pool(name="psWK", bufs=int(os.environ.get("BWK", "5")),
                         space="PSUM") as psWK,
        ):
            wq = constp.tile([DIM, DIM], BF16)
            nc.sync.dma_start(out=wq[:], in_=wq_d[:])
            wk = constp.tile([DIM, DIM], BF16)
            nc.sync.dma_start(out=wk[:], in_=wk_d[:])
            wv = constp.tile([DIM, DIM], BF16)
            nc.sync.dma_start(out=wv[:], in_=wv_d[:])
            wo2 = constp.tile([64, 2, DIM], BF16)
            nc.sync.dma_start(out=wo2[:], in_=wo_d[:])
            biasb = constp.tile([64, HEADS * N], BF16)
            nc.sync.dma_start(out=biasb[:], in_=bias_d[:])
            i98 = constp.tile([DIM, DIM], BF16)
            nc.sync.dma_start(out=i98[:], in_=i98_d[:])

            for _rep, g in [(rr, gg) for rr in range(repeats)
                            for gg in range(n_groups)]:
                r0 = g * group * P
                xs = stagep.tile([112, group, DIM], BF16, tag="xs")
                if g < 4:
                    nc.vector.memset(xs[96:112, :, :], 0.0)
                nc.gpsimd.dma_start(
                    out=xs[0:P, :, :],
                    in_=x_d[r0:r0 + group * P, :].rearrange(
                        "(p t) d -> t p d", p=group),
                )
                # grouped xbar transpose: xtg[:, p, :] = xs[:, p, :].T
                xtg = xtp.tile([DIM, group, 112], BF16, tag="xt")
                nc.sync.dma_start(out=xtg[:], in_=xs[:], transpose=True)
                fs = finp.tile([P, group, DIM], BF16, tag="fs")
                nc.scalar.memzero(fs[0:1, 0:1, 0:2])
                for q in range(quads_per_group):
                    pcd = psCD.tile([DIM, 512], F32, tag="psCD")
                    simp = pcd[:, 0:2 * HEADS * N]          # [128, 392] f32
                    xt = xtg[:, 2 * q:2 * q + 2, :]         # [128, 2, 112]
                    # ---------- qkv into one packed work bank ----------
                    # rows 0:32 q | 32:64 k | 64:113 v ; later reused for
                    # put (rows 0:49), av (rows 0:32), f (rows 0:98)
                    wk_t = psWK.tile([DIM, 512], F32, tag="pswk")
                    qt = qkp.tile([DH, HEADS, QT], BF16, tag="qt")
                    kt = qkp.tile([DH, HEADS, QT], BF16, tag="kt")
                    for r in range(2):
                        pq = wk_t[0:DH, 0:HEADS * P].rearrange(
                            "p (h t) -> p h t", h=HEADS)
                        pk = wk_t[DH:2 * DH, 0:HEADS * P].rearrange(
                            "p (h t) -> p h t", h=HEADS)
                        for h in range(HEADS):
                            nc.tensor.matmul(pq[:, h, :],
                                             lhsT=wq[:, DH * h:DH * h + DH],
                                             rhs=xt[:, r, 0:P])
                            nc.tensor.matmul(pk[:, h, :],
                                             lhsT=wk[:, DH * h:DH * h + DH],
                                             rhs=xt[:, r, 0:P])
                        if "qkc" not in SKIP:
                            nc.vector.tensor_copy(qt[:, :, P * r:P * r + P], pq[:])
                            nc.scalar.copy(kt[:, :, P * r:P * r + P], pk[:])
                        else:
                            nc.scalar.copy(qt[:, 0:1, P * r:P * r + 2],
                                           pq[:, 0:1, 0:2])
                            nc.scalar.copy(kt[:, 0:1, P * r:P * r + 2],
                                           pk[:, 0:1, 0:2])
                    # ---------- v natural ----------
                    vn = vnp.tile([N, QW, DIM], BF16, tag="vn")
                    pv = wk_t[64:64 + N, :].rearrange("p (w d) -> p w d", w=QW)
                    for w in range(QW):
                        nc.tensor.matmul(
                            pv[:, w, :],
                            lhsT=xt[:, w >> 1, N * (w & 1):N * (w & 1) + N],
                            rhs=wv[:])
                    nc.scalar.copy(vn[:], pv[:])
                    STAGE = int(os.environ.get("KSTAGE", "9"))
                    if STAGE < 3:
                        nc.scalar.copy(fs[:, 2 * q, :], pvf[0][0:P, 0:128])
                        nc.scalar.copy(fs[:, 2 * q + 1, :], pvf[1][0:P, 0:128])
                        continue
                    # ---------- sim: bias init + head matmuls ----------
                    for s in range(2):
                        for w in range(2):
                            nc.tensor.matmul(
                                simp[64 * w:64 * w + 64,
                                     HEADS * N * s:HEADS * N * (s + 1)],
                                lhsT=i98[0:64, 0:64], rhs=biasb[:],
                                start=True, stop=False,
                                skip_group_check=True)
                            for h in range(HEADS):
                                c0 = HEADS * N * s + N * h
                                t0 = P * s + N * w
                                nc.tensor.matmul(
                                    simp[64 * w:64 * w + N, c0:c0 + N],
                                    lhsT=qt[:, h, t0:t0 + N],
                                    rhs=kt[:, h, t0:t0 + N],
                                    start=False, stop=True,
                                    skip_group_check=True)
                    if STAGE < 4:
                        nc.scalar.copy(fs[:, 2 * q, :], simp[0:P, 0:128])
                        nc.scalar.copy(fs[:, 2 * q + 1, :], simp[0:P, 128:256])
                        continue
                    # ---------- softmax ----------
                    u = up.tile([DIM, 2 * HEADS * N], BF16, tag="u")
                    if "exp" not in SKIP:
                        nc.scalar.activation(u[:], simp,
                                             func=mybir.ActivationFunctionType.Exp)
                    else:
                        nc.scalar.memzero(u[:, 0:2])
                    sm = srp.tile([DIM, 2 * HEADS], F32, tag="s")
                    red_eng = (nc.gpsimd if os.environ.get("KPOOL", "0") == "1"
                               else nc.vector)
                    if "red" not in SKIP:
                        nc.vector.tensor_reduce(
                            sm[:], u[:].rearrange("p (a j) -> p a j", j=N),
                            axis=mybir.AxisListType.X, op=mybir.AluOpType.add)
                    else:
                        nc.vector.memset(sm[:], 1.0)
                    r_ = srp.tile([DIM, 2 * HEADS], F32, tag="r")
                    if "red" not in SKIP:
                        nc.vector.reciprocal(r_[:], sm[:])
                    else:
                        nc.vector.memset(r_[:], 1.0)
                    u2 = up.tile([DIM, 2 * HEADS * N], BF16, tag="u2")
                    r_b = bass.AP(
                        tensor=r_[:].tensor, offset=r_[:].offset,
                        ap=[list(r_[:].ap[0]), list(r_[:].ap[1]), [0, N]])
                    if "norm" not in SKIP:
                        red_eng.tensor_mul(
                            u2[:].rearrange("p (a j) -> p a j", j=N),
                            u[:].rearrange("p (a j) -> p a j", j=N), r_b)
                    else:
                        nc.vector.memset(u2[:, 0:2], 1.0)
                    if STAGE < 5:
                        nc.scalar.copy(fs[:, 2 * q, :], simp[0:P, 0:128])
                        nc.vector.tensor_copy(fs[:, 2 * q + 1, 0:8], r_[0:P, :])
                        continue
                    # ---------- transpose attn ----------
                    # one transpose per (stack, head): [113, 49] -> [49, 113]
                    # cols 0:49 = window A's UT, 64:113 = window B's UT
                    put = wk_t[0:N, 0:464].bitcast(BF16).rearrange(
                        "p (b c) -> p b c", b=2 * HEADS)
                    for s in range(2):
                        for h in range(HEADS):
                            b = HEADS * s + h
                            nc.tensor.transpose(
                                put[:, b, 0:113],
                                u2[0:113,
                                   HEADS * N * s + N * h:
                                   HEADS * N * s + N * h + N],
                                i98[0:113, 0:113])
                    ut = utp_pool.tile([N, 2 * HEADS, 116], BF16, tag="ut")
                    if "utc" not in SKIP:
                        nc.vector.tensor_copy(ut[:], put[:])
                    else:
                        nc.vector.memset(ut[:, 0:1, 0:2], 1.0)
                    if STAGE < 6:
                        nc.vector.tensor_copy(fs[0:N, 2 * q, 0:49], ut[:, 0, :])
                        nc.scalar.copy(fs[:, 2 * q + 1, :], simp[0:P, 0:128])
                        continue
                    # ---------- attn @ v (head-pairs stacked, base 0/32) ----------
                    pav = wk_t[0:64, 0:392].rearrange(
                        "p (hh w c) -> p hh w c", hh=2, w=QW)
                    for s in range(2):
                        for w in range(2):
                            wi = 2 * s + w
                            for h in range(HEADS):
                                nc.tensor.matmul(
                                    pav[DH * (h % 2):DH * (h % 2) + DH,
                                        h // 2, wi, :],
                                    lhsT=vn[:, wi, DH * h:DH * h + DH],
                                    rhs=ut[:, HEADS * s + h,
                                           64 * w:64 * w + N])
                    ot = otp.tile([64, 2, QW, N], BF16, tag="ot")
                    if "otc" not in SKIP:
                        nc.vector.tensor_copy(ot[:], pav[:])
                    else:
                        nc.vector.memset(ot[:, 0:1, 0:1, 0:2], 1.0)
                    if STAGE < 7:
                        nc.scalar.copy(fs[0:P, 2 * q, 64:128], simp[0:P, 0:64])
                        nc.scalar.copy(fs[:, 2 * q + 1, :], simp[0:P, 0:128])
                        continue
                    # ---------- projection per pair (K=64 head-pairs) ----------
                    for s in range(2):
                        f_ps = wk_t[0:P, 256 + 128 * s:384 + 128 * s]
                        for hh in range(2):
                            nc.tensor.matmul(
                                f_ps,
                                lhsT=ot[:, hh, 2 * s:2 * s + 2, :],
                                rhs=wo2[:, hh, :],
                                start=(hh == 0), stop=(hh == 1))
                    nc.scalar.copy(
                        fs[:, 2 * q:2 * q + 2, :],
                        wk_t[0:P, 256:512].rearrange("p (s d) -> p s d", s=2))
                nc.sync.dma_start(
                    out=out_d[r0:r0 + group * P, :].rearrange(
                        "(p t) d -> t p d", p=group),
                    in_=fs[:],
                )
    nc.finalize()
    return nc


_CACHE = {}


def _get_program(n_pairs):
    if n_pairs not in _CACHE:
        _CACHE[n_pairs] = build_program(n_pairs)
    return _CACHE[n_pairs]


def _host_inputs(W_qkv, W_out, bias_table):
    W_qkv = np.asarray(W_qkv, np.float32)
    wo = np.asarray(W_out, np.float32).reshape(2, 64, DIM).transpose(1, 0, 2)
    return {
        "wq": np.ascontiguousarray((W_qkv[:, :DIM] * SCALE)).astype(BF),
        "wk": np.ascontiguousarray(W_qkv[:, DIM:2 * DIM]).astype(BF),
        "wv": np.ascontiguousarray(W_qkv[:, 2 * DIM:]).astype(BF),
        "wo": np.ascontiguousarray(wo).astype(BF),
        "biasb": _build_bias_block(bias_table).astype(BF),
        "i98": np.eye(DIM, dtype=np.float32).astype(BF),
    }


def kernel(x, W_qkv, W_out, bias_table):
    x = np.asarray(x, np.float32)
    shp = x.shape
    xf = np.ascontiguousarray(x.reshape(-1, DIM))
    tok = xf.shape[0]
    per = tok // NCORES
    n_pairs = per // P
    assert per % P == 0
    nc = _get_program(n_pairs)
    consts = _host_inputs(W_qkv, W_out, bias_table)
    in_maps = []
    for c in range(NCORES):
        m = {"x": np.ascontiguousarray(xf[c * per:(c + 1) * per])}
        m.update(consts)
        in_maps.append(m)
    res = run_bass_kernel_spmd(nc, in_maps, list(range(NCORES)))
    outs = [res.results[c]["out"] for c in range(NCORES)]
    return np.concatenate(outs, 0).reshape(shp).astype(np.float32)

